# revision 17
# baseline (speedup 1.0000x reference)
"""Decoder-only attention block (QKV proj + MHA + out proj) on 8 TRN2 cores.

Sharding: core c -> (batch b = c//4, head-group g = c%4). Tensor-parallel over
heads (4 of 16 heads per core), data-parallel over batch (2). Each core
computes a partial c_proj over its 512 input features; host reduces the 4
partials per batch and adds biases.

Self-contained: hardcodes B=2, S=2048, D=2048, H=16.
"""

import os

import numpy as np

NPF16 = np.float16

import concourse.bass as bass
import concourse.bacc as bacc
import concourse.tile as tile
from concourse import mybir
import concourse.bass_utils as bass_utils
import concourse.bass_isa as bass_isa
from concourse.bass_interp import get_hw_module

B, S, D = 2, 2048, 2048
H, DH = 16, 128
N_CORES = 8
HL = H // 4            # 4 heads per core
FL = HL * DH           # 512 local features per core
KT = D // 128          # 16 contraction tiles
TT = S // 128          # 16 token tiles
QB = S // 512          # 4 token blocks
SCALE = 1.0 / float(np.sqrt(DH))

F16 = mybir.dt.float16
F32 = mybir.dt.float32

# Stash of the last BassKernelResults (for the local test harness only).
LAST_RESULTS = None
_PROG_CACHE = {}


def _build_program(use_mask):

    nc = bacc.Bacc("TRN2", target_bir_lowering=False, debug=False,
                   num_devices=N_CORES)

    xt_d = nc.dram_tensor("xt", [D, S], F16, kind="ExternalInput")
    wqk_d = nc.dram_tensor("wqk", [D, 2 * FL], F16, kind="ExternalInput")
    wv_d = nc.dram_tensor("wv", [D, FL], F16, kind="ExternalInput")
    wp_d = nc.dram_tensor("wp", [FL, D], F16, kind="ExternalInput")
    bqk_d = nc.dram_tensor("bqk", [128, 8], F32, kind="ExternalInput")
    kb_d = nc.dram_tensor("kb", [128, KT], F32, kind="ExternalInput")
    out_d = nc.dram_tensor("out", [S, D], F32, kind="ExternalOutput")

    xt_ap, wqk_ap, wv_ap, wp_ap = xt_d.ap(), wqk_d.ap(), wv_d.ap(), wp_d.ap()
    bqk_ap, kb_ap, out_ap = bqk_d.ap(), kb_d.ap(), out_d.ap()

    with tile.TileContext(nc) as tc, tc.tile_pool(name="pers", bufs=1) as pers:
        # ---- persistent tiles (live across phases) ----
        qt = [pers.tile([128, S], F16, tag=f"qt{h}", name=f"qt{h}") for h in range(HL)]
        ktt = [pers.tile([128, S], F16, tag=f"kt{h}", name=f"ktt{h}") for h in range(HL)]
        ot = [pers.tile([128, S], F16, tag=f"ot{h}", name=f"ot{h}") for h in range(HL)]
        vaug = [[pers.tile([128, DH], F16, tag=f"v{t}_{h}", name=f"v{t}_{h}")
                 for h in range(HL)] for t in range(TT)]
        wp_sb = [pers.tile([128, D], F16, tag=f"wp{h}", name=f"wp{h}") for h in range(HL)]
        bqk_sb = pers.tile([128, 8], F32, tag="bqk", name="bqk_sb")
        kb_sb = pers.tile([128, KT], F32, tag="kb", name="kb_sb")

        nc.sync.dma_start(bqk_sb[:], bqk_ap[:])
        nc.sync.dma_start(kb_sb[:], kb_ap[:])

        # ---- phase 1: QKV projection ----
        with (
            tc.tile_pool(name="p1in", bufs=1) as p1in,
            tc.tile_pool(name="p1ps", bufs=1, space="PSUM") as p1ps,
        ):
            xt_sb, wqk_sb, wv_sb = [], [], []
            for kt in range(KT):
                w = p1in.tile([128, 2 * FL], F16, tag=f"wqk{kt}", name=f"wqk{kt}")
                nc.sync.dma_start(w[:], wqk_ap[kt * 128:(kt + 1) * 128, :])
                wqk_sb.append(w)
                x = p1in.tile([128, S], F16, tag=f"xt{kt}", name=f"xt{kt}")
                nc.sync.dma_start(x[:], xt_ap[kt * 128:(kt + 1) * 128, :])
                xt_sb.append(x)
            for kt in range(KT):
                v = p1in.tile([128, FL], F16, tag=f"wv{kt}", name=f"wv{kt}")
                nc.sync.dma_start(v[:], wv_ap[kt * 128:(kt + 1) * 128, :])
                wv_sb.append(v)
            for h in range(HL):
                nc.sync.dma_start(wp_sb[h][:], wp_ap[h * 128:(h + 1) * 128, :])

            # Q^T and K^T: [feat, tok], weight chunks stationary
            for f in range(8):
                ps = [None] * QB
                for kt in range(KT):
                    for tb in range(QB):
                        if kt == 0:
                            ps[tb] = p1ps.tile([128, 512], F32, tag=f"psqk{tb}", bufs=1, name=f"psqk{tb}")
                        nc.tensor.matmul(
                            ps[tb][:],
                            wqk_sb[kt][:, f * 128:(f + 1) * 128],
                            xt_sb[kt][:, tb * 512:(tb + 1) * 512],
                            start=(kt == 0), stop=(kt == KT - 1),
                            skip_group_check=True,
                        )
                dest = qt[f] if f < HL else ktt[f - HL]
                for tb in range(QB):
                    nc.scalar.add(dest[:, tb * 512:(tb + 1) * 512], ps[tb][:],
                                  bqk_sb[:, f:f + 1])

            # V: [tok, feat], x^T chunks stationary
            for t in range(TT):
                psv = p1ps.tile([128, FL], F32, tag="psv", bufs=2, name="psv")
                for kt in range(KT):
                    nc.tensor.matmul(
                        psv[:],
                        xt_sb[kt][:, t * 128:(t + 1) * 128],
                        wv_sb[kt][:],
                        start=(kt == 0), stop=(kt == KT - 1),
                    )
                for h in range(HL):
                    nc.vector.tensor_copy(vaug[t][h][:],
                                          psv[:, h * 128:(h + 1) * 128])

        # ---- phase 2: attention per (token-block, head) ----
        # scores^T [ktok, qtok] pairs (2 k-tiles per 2-bank psum) -> one exp
        # [128,1024] on ScalarE -> E^T (f16). Softmax denominator: DVE
        # accumulates sum over the 16 E tiles, GpSimd all-reduces across
        # partitions, DVE reciprocal. O^T = V^T @ A^T with V chunks
        # stationary; normalization fused into PSUM evacuation.
        with (
            tc.tile_pool(name="p2", bufs=1) as p2,
            tc.tile_pool(name="p2ps", bufs=1, space="PSUM") as p2ps,
        ):
            for qb in range(QB):
                for h in range(HL):
                    e_tiles = []
                    for p in range(KT // 2):
                        pss = p2ps.tile([128, 1024], F32, tag="pss", bufs=2,
                                        name="pss")
                        for half in range(2):
                            kt = 2 * p + half
                            nc.tensor.matmul(
                                pss[:, half * 512:(half + 1) * 512],
                                ktt[h][:, kt * 128:(kt + 1) * 128],
                                qt[h][:, qb * 512:(qb + 1) * 512],
                                start=True, stop=True,
                            )
                        e = p2.tile([128, 1024], F16, tag=f"e{p}", bufs=3,
                                    name=f"e{p}")
                        nc.scalar.activation(
                            e[:], pss[:], mybir.ActivationFunctionType.Exp,
                            scale=SCALE,
                        )
                        if use_mask:
                            for half in range(2):
                                kt = 2 * p + half
                                sl = e[:, half * 512:(half + 1) * 512]
                                nc.vector.tensor_scalar_mul(
                                    sl, sl, kb_sb[:, kt:kt + 1])
                        e_tiles.append(e)
                    # denominator: f16 pairwise level (DVE 2x mode), then f32
                    l1 = []
                    for i in range(4):
                        t_ = p2.tile([128, 1024], F16, tag=f"l1_{i}", bufs=2,
                                     name=f"l1_{i}")
                        nc.vector.tensor_add(t_[:], e_tiles[2 * i][:],
                                             e_tiles[2 * i + 1][:])
                        l1.append(t_)
                    l2 = []
                    for i in range(2):
                        t_ = p2.tile([128, 1024], F32, tag=f"l2_{i}", bufs=2,
                                     name=f"l2_{i}")
                        nc.vector.tensor_add(t_[:], l1[2 * i][:],
                                             l1[2 * i + 1][:])
                        l2.append(t_)
                    l3 = p2.tile([128, 1024], F32, tag="l3", bufs=2, name="l3")
                    nc.vector.tensor_add(l3[:], l2[0][:], l2[1][:])
                    dn = p2.tile([128, 512], F16, tag="dn", bufs=3, name="dn")
                    nc.vector.tensor_add(dn[:], l3[:, 0:512], l3[:, 512:1024])
                    dnr = p2.tile([128, 512], F32, tag="dnr", bufs=3,
                                  name="dnr")
                    nc.gpsimd.partition_all_reduce(
                        dnr[:], dn[:], channels=128,
                        reduce_op=bass_isa.ReduceOp.add)
                    rcp = p2.tile([128, 512], F32, tag="rcp", bufs=3,
                                  name="rcp")
                    nc.vector.reciprocal_approx_fast(rcp[:], dnr[:])
                    psot = p2ps.tile([128, 512], F32, tag="psot", bufs=2,
                                     name="psot")
                    for p in range(KT // 2):
                        for half in range(2):
                            kt = 2 * p + half
                            nc.tensor.matmul(
                                psot[:],
                                vaug[kt][h][:],
                                e_tiles[p][:, half * 512:(half + 1) * 512],
                                start=(kt == 0), stop=(kt == KT - 1),
                            )
                    nc.vector.tensor_mul(ot[h][:, qb * 512:(qb + 1) * 512],
                                         psot[:], rcp[:])

                # partial c_proj for this token block (all 4 heads ready)
                for t in range(qb * 4, qb * 4 + 4):
                    for nb in range(4):
                        psp = p2ps.tile([128, 512], F32, tag=f"psp{nb % 2}",
                                        bufs=1, name=f"psp{nb % 2}")
                        for h in range(HL):
                            nc.tensor.matmul(
                                psp[:],
                                ot[h][:, t * 128:(t + 1) * 128],
                                wp_sb[h][:, nb * 512:(nb + 1) * 512],
                                start=(h == 0), stop=(h == HL - 1),
                                skip_group_check=True,
                            )
                        st = p2.tile([128, 512], F32, tag="stage", bufs=6,
                                     name="stage")
                        if nb % 2 == 0:
                            nc.scalar.copy(st[:], psp[:])
                        else:
                            nc.vector.tensor_copy(st[:], psp[:])
                        nc.sync.dma_start(
                            out_ap[t * 128:(t + 1) * 128,
                                   nb * 512:(nb + 1) * 512], st[:])

    nc.compile()
    nc.m = get_hw_module(nc.m)
    return nc


def kernel(hidden_states, attention_mask, w_attn, b_attn, w_proj, b_proj):
    global LAST_RESULTS
    hidden_states = np.asarray(hidden_states, dtype=np.float32)
    attention_mask = np.asarray(attention_mask, dtype=np.float32)
    w_attn = np.asarray(w_attn, dtype=np.float32)
    b_attn = np.asarray(b_attn, dtype=np.float32)
    w_proj = np.asarray(w_proj, dtype=np.float32)
    b_proj = np.asarray(b_proj, dtype=np.float32)

    use_mask = bool((attention_mask != 1.0).any())
    key = ("prog", use_mask)
    if key not in _PROG_CACHE:
        _PROG_CACHE[key] = _build_program(use_mask)
    nc = _PROG_CACHE[key]

    in_maps = []
    for c in range(N_CORES):
        b, g = divmod(c, 4)
        xt = np.ascontiguousarray(hidden_states[b].T).astype(NPF16)
        wq = w_attn[:, g * FL:(g + 1) * FL]
        wk = w_attn[:, D + g * FL:D + (g + 1) * FL]
        wv = w_attn[:, 2 * D + g * FL:2 * D + (g + 1) * FL]
        wqk = np.concatenate([wq, wk], axis=1).astype(NPF16)
        wp = np.ascontiguousarray(w_proj[g * FL:(g + 1) * FL, :]).astype(NPF16)
        bq = b_attn[g * FL:(g + 1) * FL]
        bk = b_attn[D + g * FL:D + (g + 1) * FL]
        bqk = np.ascontiguousarray(
            np.concatenate([bq, bk]).reshape(8, 128).T).astype(np.float32)
        kb = np.ascontiguousarray(
            attention_mask[b].reshape(KT, 128).T).astype(np.float32)
        in_maps.append({
            "xt": xt,
            "wqk": wqk,
            "wv": np.ascontiguousarray(wv).astype(NPF16),
            "wp": wp,
            "bqk": bqk,
            "kb": kb,
        })

    if not os.environ.get("KERNEL_ALLOW_TRACE"):
        os.environ["BASS_NEVER_TRACE"] = "1"
    res = bass_utils.run_bass_kernel_spmd(nc, in_maps, list(range(N_CORES)))
    LAST_RESULTS = res

    # host reduce: sum the 4 head-group partials per batch, add biases.
    # V-bias contribution: rows of A sum to 1, so each core's O gains b_v
    # per row; through c_proj that's a constant row b_v @ w_proj_slice.
    out = np.zeros((B, S, D), dtype=np.float32)
    for c in range(N_CORES):
        b, g = divmod(c, 4)
        out[b] += res.results[c]["out"]
    bias_row = b_proj.astype(np.float64).copy()
    for g in range(4):
        bv = b_attn[2 * D + g * FL:2 * D + (g + 1) * FL].astype(np.float64)
        bias_row += bv @ w_proj[g * FL:(g + 1) * FL, :].astype(np.float64)
    out += bias_row.astype(np.float32)[None, None, :]
    return out


# revision 18
# speedup vs baseline: 1.0283x; 1.0283x over previous
"""Decoder-only attention block (QKV proj + MHA + out proj) on 8 TRN2 cores.

Sharding: core c -> (batch b = c//4, head-group g = c%4). Tensor-parallel over
heads (4 of 16 heads per core), data-parallel over batch (2). Each core
computes a partial c_proj over its 512 input features; host reduces the 4
partials per batch and adds biases.

Self-contained: hardcodes B=2, S=2048, D=2048, H=16.
"""

import os

import numpy as np

NPF16 = np.float16

import concourse.bass as bass
import concourse.bacc as bacc
import concourse.tile as tile
from concourse import mybir
import concourse.bass_utils as bass_utils
import concourse.bass_isa as bass_isa
from concourse.bass_interp import get_hw_module

B, S, D = 2, 2048, 2048
H, DH = 16, 128
N_CORES = 8
HL = H // 4            # 4 heads per core
FL = HL * DH           # 512 local features per core
KT = D // 128          # 16 contraction tiles
TT = S // 128          # 16 token tiles
QB = S // 512          # 4 token blocks
SCALE = 1.0 / float(np.sqrt(DH))

F16 = mybir.dt.float16
F32 = mybir.dt.float32

# Stash of the last BassKernelResults (for the local test harness only).
LAST_RESULTS = None
_PROG_CACHE = {}


def _build_program(use_mask):

    nc = bacc.Bacc("TRN2", target_bir_lowering=False, debug=False,
                   num_devices=N_CORES)

    xt_d = nc.dram_tensor("xt", [D, S], F16, kind="ExternalInput")
    wqk_d = nc.dram_tensor("wqk", [D, 2 * FL], F16, kind="ExternalInput")
    wv_d = nc.dram_tensor("wv", [D, FL], F16, kind="ExternalInput")
    wp_d = nc.dram_tensor("wp", [FL, D], F16, kind="ExternalInput")
    bqk_d = nc.dram_tensor("bqk", [128, 8], F32, kind="ExternalInput")
    kb_d = nc.dram_tensor("kb", [128, KT], F32, kind="ExternalInput")
    out_d = nc.dram_tensor("out", [S, D], F32, kind="ExternalOutput")

    xt_ap, wqk_ap, wv_ap, wp_ap = xt_d.ap(), wqk_d.ap(), wv_d.ap(), wp_d.ap()
    bqk_ap, kb_ap, out_ap = bqk_d.ap(), kb_d.ap(), out_d.ap()

    with tile.TileContext(nc) as tc, tc.tile_pool(name="pers", bufs=1) as pers:
        # ---- persistent tiles (live across phases) ----
        qt = [pers.tile([128, S], F16, tag=f"qt{h}", name=f"qt{h}") for h in range(HL)]
        ktt = [pers.tile([128, S], F16, tag=f"kt{h}", name=f"ktt{h}") for h in range(HL)]
        ot = [pers.tile([128, S], F16, tag=f"ot{h}", name=f"ot{h}") for h in range(HL)]
        vaug = [[pers.tile([128, DH], F16, tag=f"v{t}_{h}", name=f"v{t}_{h}")
                 for h in range(HL)] for t in range(TT)]
        wp_sb = [pers.tile([128, D], F16, tag=f"wp{h}", name=f"wp{h}") for h in range(HL)]
        bqk_sb = pers.tile([128, 8], F32, tag="bqk", name="bqk_sb")
        kb_sb = pers.tile([128, KT], F32, tag="kb", name="kb_sb")

        nc.sync.dma_start(bqk_sb[:], bqk_ap[:])
        nc.sync.dma_start(kb_sb[:], kb_ap[:])

        # ---- phase 1: QKV projection ----
        with (
            tc.tile_pool(name="p1in", bufs=1) as p1in,
            tc.tile_pool(name="p1ps", bufs=1, space="PSUM") as p1ps,
        ):
            xt_sb, wqk_sb, wv_sb = [], [], []
            for kt in range(KT):
                w = p1in.tile([128, 2 * FL], F16, tag=f"wqk{kt}", name=f"wqk{kt}")
                nc.sync.dma_start(w[:], wqk_ap[kt * 128:(kt + 1) * 128, :])
                wqk_sb.append(w)
                x = p1in.tile([128, S], F16, tag=f"xt{kt}", name=f"xt{kt}")
                nc.sync.dma_start(x[:], xt_ap[kt * 128:(kt + 1) * 128, :])
                xt_sb.append(x)
            for kt in range(KT):
                v = p1in.tile([128, FL], F16, tag=f"wv{kt}", name=f"wv{kt}")
                nc.sync.dma_start(v[:], wv_ap[kt * 128:(kt + 1) * 128, :])
                wv_sb.append(v)
            for h in range(HL):
                nc.sync.dma_start(wp_sb[h][:], wp_ap[h * 128:(h + 1) * 128, :])

            # Q^T and K^T: [feat, tok], weight chunks stationary
            for f in range(8):
                ps = [None] * QB
                for kt in range(KT):
                    for tb in range(QB):
                        if kt == 0:
                            ps[tb] = p1ps.tile([128, 512], F32, tag=f"psqk{tb}", bufs=1, name=f"psqk{tb}")
                        nc.tensor.matmul(
                            ps[tb][:],
                            wqk_sb[kt][:, f * 128:(f + 1) * 128],
                            xt_sb[kt][:, tb * 512:(tb + 1) * 512],
                            start=(kt == 0), stop=(kt == KT - 1),
                            skip_group_check=True,
                        )
                dest = qt[f] if f < HL else ktt[f - HL]
                for tb in range(QB):
                    nc.scalar.add(dest[:, tb * 512:(tb + 1) * 512], ps[tb][:],
                                  bqk_sb[:, f:f + 1])

            # V: [tok, feat], x^T chunks stationary
            for t in range(TT):
                psv = p1ps.tile([128, FL], F32, tag="psv", bufs=2, name="psv")
                for kt in range(KT):
                    nc.tensor.matmul(
                        psv[:],
                        xt_sb[kt][:, t * 128:(t + 1) * 128],
                        wv_sb[kt][:],
                        start=(kt == 0), stop=(kt == KT - 1),
                    )
                for h in range(HL):
                    nc.vector.tensor_copy(vaug[t][h][:],
                                          psv[:, h * 128:(h + 1) * 128])

        # ---- phase 2: attention per (token-block, head) ----
        # scores^T [ktok, qtok] pairs (2 k-tiles per 2-bank psum) -> one exp
        # [128,1024] on ScalarE -> E^T (f16). Softmax denominator: DVE
        # accumulates sum over the 16 E tiles, GpSimd all-reduces across
        # partitions, DVE reciprocal. O^T = V^T @ A^T with V chunks
        # stationary; normalization fused into PSUM evacuation.
        with (
            tc.tile_pool(name="p2", bufs=1) as p2,
            tc.tile_pool(name="p2ps", bufs=1, space="PSUM") as p2ps,
        ):
            for qb in range(QB):
                for h in range(HL):
                    e_tiles = []
                    for p in range(KT // 2):
                        pss = p2ps.tile([128, 1024], F32, tag="pss", bufs=2,
                                        name="pss")
                        for half in range(2):
                            kt = 2 * p + half
                            nc.tensor.matmul(
                                pss[:, half * 512:(half + 1) * 512],
                                ktt[h][:, kt * 128:(kt + 1) * 128],
                                qt[h][:, qb * 512:(qb + 1) * 512],
                                start=True, stop=True,
                            )
                        e = p2.tile([128, 1024], F16, tag=f"e{p}", bufs=3,
                                    name=f"e{p}")
                        nc.scalar.activation(
                            e[:], pss[:], mybir.ActivationFunctionType.Exp,
                            scale=SCALE,
                        )
                        if use_mask:
                            for half in range(2):
                                kt = 2 * p + half
                                sl = e[:, half * 512:(half + 1) * 512]
                                nc.vector.tensor_scalar_mul(
                                    sl, sl, kb_sb[:, kt:kt + 1])
                        e_tiles.append(e)
                    # denominator: f16 pairwise level (DVE 2x mode), then f32
                    l1 = []
                    for i in range(4):
                        t_ = p2.tile([128, 1024], F16, tag=f"l1_{i}", bufs=2,
                                     name=f"l1_{i}")
                        nc.vector.tensor_add(t_[:], e_tiles[2 * i][:],
                                             e_tiles[2 * i + 1][:])
                        l1.append(t_)
                    l2 = []
                    for i in range(2):
                        t_ = p2.tile([128, 1024], F32, tag=f"l2_{i}", bufs=2,
                                     name=f"l2_{i}")
                        nc.vector.tensor_add(t_[:], l1[2 * i][:],
                                             l1[2 * i + 1][:])
                        l2.append(t_)
                    l3 = p2.tile([128, 1024], F32, tag="l3", bufs=2, name="l3")
                    nc.vector.tensor_add(l3[:], l2[0][:], l2[1][:])
                    dn = p2.tile([128, 512], F32, tag="dn", bufs=3, name="dn")
                    nc.vector.tensor_add(dn[:], l3[:, 0:512], l3[:, 512:1024])
                    dnr = p2.tile([128, 512], F32, tag="dnr", bufs=3,
                                  name="dnr")
                    nc.gpsimd.partition_all_reduce(
                        dnr[:], dn[:], channels=128,
                        reduce_op=bass_isa.ReduceOp.add)
                    rcp = p2.tile([128, 512], F32, tag="rcp", bufs=3,
                                  name="rcp")
                    nc.vector.reciprocal_approx_fast(rcp[:], dnr[:])
                    psot = p2ps.tile([128, 512], F32, tag="psot", bufs=2,
                                     name="psot")
                    for p in range(KT // 2):
                        for half in range(2):
                            kt = 2 * p + half
                            nc.tensor.matmul(
                                psot[:],
                                vaug[kt][h][:],
                                e_tiles[p][:, half * 512:(half + 1) * 512],
                                start=(kt == 0), stop=(kt == KT - 1),
                            )
                    nc.vector.tensor_mul(ot[h][:, qb * 512:(qb + 1) * 512],
                                         psot[:], rcp[:])

                # partial c_proj for this token block (all 4 heads ready)
                for t in range(qb * 4, qb * 4 + 4):
                    for nb in range(4):
                        psp = p2ps.tile([128, 512], F32, tag=f"psp{nb % 2}",
                                        bufs=1, name=f"psp{nb % 2}")
                        for h in range(HL):
                            nc.tensor.matmul(
                                psp[:],
                                ot[h][:, t * 128:(t + 1) * 128],
                                wp_sb[h][:, nb * 512:(nb + 1) * 512],
                                start=(h == 0), stop=(h == HL - 1),
                                skip_group_check=True,
                            )
                        st = p2.tile([128, 512], F32, tag="stage", bufs=6,
                                     name="stage")
                        nc.scalar.copy(st[:], psp[:])
                        nc.sync.dma_start(
                            out_ap[t * 128:(t + 1) * 128,
                                   nb * 512:(nb + 1) * 512], st[:])

    nc.compile()
    nc.m = get_hw_module(nc.m)
    return nc


def kernel(hidden_states, attention_mask, w_attn, b_attn, w_proj, b_proj):
    global LAST_RESULTS
    hidden_states = np.asarray(hidden_states, dtype=np.float32)
    attention_mask = np.asarray(attention_mask, dtype=np.float32)
    w_attn = np.asarray(w_attn, dtype=np.float32)
    b_attn = np.asarray(b_attn, dtype=np.float32)
    w_proj = np.asarray(w_proj, dtype=np.float32)
    b_proj = np.asarray(b_proj, dtype=np.float32)

    use_mask = bool((attention_mask != 1.0).any())
    key = ("prog", use_mask)
    if key not in _PROG_CACHE:
        _PROG_CACHE[key] = _build_program(use_mask)
    nc = _PROG_CACHE[key]

    in_maps = []
    for c in range(N_CORES):
        b, g = divmod(c, 4)
        xt = np.ascontiguousarray(hidden_states[b].T).astype(NPF16)
        wq = w_attn[:, g * FL:(g + 1) * FL]
        wk = w_attn[:, D + g * FL:D + (g + 1) * FL]
        wv = w_attn[:, 2 * D + g * FL:2 * D + (g + 1) * FL]
        wqk = np.concatenate([wq, wk], axis=1).astype(NPF16)
        wp = np.ascontiguousarray(w_proj[g * FL:(g + 1) * FL, :]).astype(NPF16)
        bq = b_attn[g * FL:(g + 1) * FL]
        bk = b_attn[D + g * FL:D + (g + 1) * FL]
        bqk = np.ascontiguousarray(
            np.concatenate([bq, bk]).reshape(8, 128).T).astype(np.float32)
        kb = np.ascontiguousarray(
            attention_mask[b].reshape(KT, 128).T).astype(np.float32)
        in_maps.append({
            "xt": xt,
            "wqk": wqk,
            "wv": np.ascontiguousarray(wv).astype(NPF16),
            "wp": wp,
            "bqk": bqk,
            "kb": kb,
        })

    if not os.environ.get("KERNEL_ALLOW_TRACE"):
        os.environ["BASS_NEVER_TRACE"] = "1"
    res = bass_utils.run_bass_kernel_spmd(nc, in_maps, list(range(N_CORES)))
    LAST_RESULTS = res

    # host reduce: sum the 4 head-group partials per batch, add biases.
    # V-bias contribution: rows of A sum to 1, so each core's O gains b_v
    # per row; through c_proj that's a constant row b_v @ w_proj_slice.
    out = np.zeros((B, S, D), dtype=np.float32)
    for c in range(N_CORES):
        b, g = divmod(c, 4)
        out[b] += res.results[c]["out"]
    bias_row = b_proj.astype(np.float64).copy()
    for g in range(4):
        bv = b_attn[2 * D + g * FL:2 * D + (g + 1) * FL].astype(np.float64)
        bias_row += bv @ w_proj[g * FL:(g + 1) * FL, :].astype(np.float64)
    out += bias_row.astype(np.float32)[None, None, :]
    return out


# revision 21
# speedup vs baseline: 1.0831x; 1.0533x over previous
"""Decoder-only attention block (QKV proj + MHA + out proj) on 8 TRN2 cores.

Sharding: core c -> (batch b = c//4, head-group g = c%4). Tensor-parallel over
heads (4 of 16 heads per core), data-parallel over batch (2). Each core
computes a partial c_proj over its 512 input features; host reduces the 4
partials per batch and adds biases.

Self-contained: hardcodes B=2, S=2048, D=2048, H=16.
"""

import os

import numpy as np

NPF16 = np.float16

import concourse.bass as bass
import concourse.bacc as bacc
import concourse.tile as tile
from concourse import mybir
import concourse.bass_utils as bass_utils
import concourse.bass_isa as bass_isa
from concourse.bass_interp import get_hw_module

B, S, D = 2, 2048, 2048
H, DH = 16, 128
N_CORES = 8
HL = H // 4            # 4 heads per core
FL = HL * DH           # 512 local features per core
KT = D // 128          # 16 contraction tiles
TT = S // 128          # 16 token tiles
QB = S // 512          # 4 token blocks
SCALE = 1.0 / float(np.sqrt(DH))

F16 = mybir.dt.float16
F32 = mybir.dt.float32

# Stash of the last BassKernelResults (for the local test harness only).
LAST_RESULTS = None
_PROG_CACHE = {}


def _build_program(use_mask):

    nc = bacc.Bacc("TRN2", target_bir_lowering=False, debug=False,
                   num_devices=N_CORES)

    xt_d = nc.dram_tensor("xt", [D, S], F16, kind="ExternalInput")
    wqk_d = nc.dram_tensor("wqk", [D, 2 * FL], F16, kind="ExternalInput")
    wv_d = nc.dram_tensor("wv", [D, FL], F16, kind="ExternalInput")
    wp_d = nc.dram_tensor("wp", [FL, D], F16, kind="ExternalInput")
    bqk_d = nc.dram_tensor("bqk", [128, 8], F32, kind="ExternalInput")
    kb_d = nc.dram_tensor("kb", [128, KT], F32, kind="ExternalInput")
    out_d = nc.dram_tensor("out", [S, D], F32, kind="ExternalOutput")

    xt_ap, wqk_ap, wv_ap, wp_ap = xt_d.ap(), wqk_d.ap(), wv_d.ap(), wp_d.ap()
    bqk_ap, kb_ap, out_ap = bqk_d.ap(), kb_d.ap(), out_d.ap()

    with tile.TileContext(nc) as tc, tc.tile_pool(name="pers", bufs=1) as pers:
        # ---- persistent tiles (live across phases) ----
        qt = [pers.tile([128, S], F16, tag=f"qt{h}", name=f"qt{h}") for h in range(HL)]
        ktt = [pers.tile([128, S], F16, tag=f"kt{h}", name=f"ktt{h}") for h in range(HL)]
        ot = [pers.tile([128, S], F16, tag=f"ot{h}", name=f"ot{h}") for h in range(HL)]
        vaug = [[pers.tile([128, DH], F16, tag=f"v{t}_{h}", name=f"v{t}_{h}")
                 for h in range(HL)] for t in range(TT)]
        wp_sb = [pers.tile([128, D], F16, tag=f"wp{h}", name=f"wp{h}") for h in range(HL)]
        bqk_sb = pers.tile([128, 8], F32, tag="bqk", name="bqk_sb")
        kb_sb = pers.tile([128, KT], F32, tag="kb", name="kb_sb")

        nc.sync.dma_start(bqk_sb[:], bqk_ap[:])
        nc.sync.dma_start(kb_sb[:], kb_ap[:])

        # ---- phase 1: QKV projection ----
        with (
            tc.tile_pool(name="p1in", bufs=1) as p1in,
            tc.tile_pool(name="p1ps", bufs=1, space="PSUM") as p1ps,
        ):
            xt_sb, wqk_sb, wv_sb = [], [], []
            for kt in range(KT):
                w = p1in.tile([128, 2 * FL], F16, tag=f"wqk{kt}", name=f"wqk{kt}")
                nc.sync.dma_start(w[:], wqk_ap[kt * 128:(kt + 1) * 128, :])
                wqk_sb.append(w)
                x = p1in.tile([128, S], F16, tag=f"xt{kt}", name=f"xt{kt}")
                nc.sync.dma_start(x[:], xt_ap[kt * 128:(kt + 1) * 128, :])
                xt_sb.append(x)
            for kt in range(KT):
                v = p1in.tile([128, FL], F16, tag=f"wv{kt}", name=f"wv{kt}")
                nc.sync.dma_start(v[:], wv_ap[kt * 128:(kt + 1) * 128, :])
                wv_sb.append(v)
            for h in range(HL):
                nc.sync.dma_start(wp_sb[h][:], wp_ap[h * 128:(h + 1) * 128, :])

            # Q^T and K^T: [feat, tok], weight chunks stationary
            for f in range(8):
                ps = [None] * QB
                for kt in range(KT):
                    for tb in range(QB):
                        if kt == 0:
                            ps[tb] = p1ps.tile([128, 512], F32, tag=f"psqk{tb}", bufs=1, name=f"psqk{tb}")
                        nc.tensor.matmul(
                            ps[tb][:],
                            wqk_sb[kt][:, f * 128:(f + 1) * 128],
                            xt_sb[kt][:, tb * 512:(tb + 1) * 512],
                            start=(kt == 0), stop=(kt == KT - 1),
                            skip_group_check=True,
                        )
                dest = qt[f] if f < HL else ktt[f - HL]
                for tb in range(QB):
                    nc.scalar.add(dest[:, tb * 512:(tb + 1) * 512], ps[tb][:],
                                  bqk_sb[:, f:f + 1])

            # V: [tok, feat], x^T chunks stationary
            for t in range(TT):
                psv = p1ps.tile([128, FL], F32, tag="psv", bufs=2, name="psv")
                for kt in range(KT):
                    nc.tensor.matmul(
                        psv[:],
                        xt_sb[kt][:, t * 128:(t + 1) * 128],
                        wv_sb[kt][:],
                        start=(kt == 0), stop=(kt == KT - 1),
                    )
                for h in range(HL):
                    nc.vector.tensor_copy(vaug[t][h][:],
                                          psv[:, h * 128:(h + 1) * 128])

        # ---- phase 2: attention per (token-block, head) ----
        # scores^T [ktok, qtok] pairs (2 k-tiles per 2-bank psum) -> one exp
        # [128,1024] on ScalarE -> E^T (f16). Softmax denominator: DVE
        # accumulates sum over the 16 E tiles, GpSimd all-reduces across
        # partitions, DVE reciprocal. O^T = V^T @ A^T with V chunks
        # stationary; normalization fused into PSUM evacuation.
        with (
            tc.tile_pool(name="p2", bufs=1) as p2,
            tc.tile_pool(name="p2ps", bufs=1, space="PSUM") as p2ps,
        ):
            for qb in range(QB):
                for h in range(HL):
                    e_tiles = []
                    for p in range(KT // 2):
                        pss = p2ps.tile([128, 1024], F32, tag="pss", bufs=2,
                                        name="pss")
                        for half in range(2):
                            kt = 2 * p + half
                            nc.tensor.matmul(
                                pss[:, half * 512:(half + 1) * 512],
                                ktt[h][:, kt * 128:(kt + 1) * 128],
                                qt[h][:, qb * 512:(qb + 1) * 512],
                                start=True, stop=True,
                            )
                        e = p2.tile([128, 1024], F16, tag=f"e{p}", bufs=3,
                                    name=f"e{p}")
                        nc.scalar.activation(
                            e[:], pss[:], mybir.ActivationFunctionType.Exp,
                            scale=SCALE,
                        )
                        if use_mask:
                            for half in range(2):
                                kt = 2 * p + half
                                sl = e[:, half * 512:(half + 1) * 512]
                                nc.vector.tensor_scalar_mul(
                                    sl, sl, kb_sb[:, kt:kt + 1])
                        e_tiles.append(e)
                    # denominator: f16 pairwise level (DVE 2x mode), then f32
                    l1 = []
                    for i in range(4):
                        t_ = p2.tile([128, 1024], F16, tag=f"l1_{i}", bufs=2,
                                     name=f"l1_{i}")
                        nc.vector.tensor_add(t_[:], e_tiles[2 * i][:],
                                             e_tiles[2 * i + 1][:])
                        l1.append(t_)
                    l2 = []
                    for i in range(2):
                        t_ = p2.tile([128, 1024], F32, tag=f"l2_{i}", bufs=2,
                                     name=f"l2_{i}")
                        nc.vector.tensor_add(t_[:], l1[2 * i][:],
                                             l1[2 * i + 1][:])
                        l2.append(t_)
                    l3 = p2.tile([128, 1024], F32, tag="l3", bufs=2, name="l3")
                    nc.vector.tensor_add(l3[:], l2[0][:], l2[1][:])
                    dn = p2.tile([128, 512], F16, tag="dn", bufs=3, name="dn")
                    nc.vector.tensor_add(dn[:], l3[:, 0:512], l3[:, 512:1024])
                    dnr = p2.tile([128, 512], F32, tag="dnr", bufs=3,
                                  name="dnr")
                    nc.gpsimd.partition_all_reduce(
                        dnr[:], dn[:], channels=128,
                        reduce_op=bass_isa.ReduceOp.add)
                    rcp = p2.tile([128, 512], F32, tag="rcp", bufs=3,
                                  name="rcp")
                    nc.vector.reciprocal_approx_fast(rcp[:], dnr[:])
                    psot = p2ps.tile([128, 512], F32, tag="psot", bufs=2,
                                     name="psot")
                    for p in range(KT // 2):
                        for half in range(2):
                            kt = 2 * p + half
                            nc.tensor.matmul(
                                psot[:],
                                vaug[kt][h][:],
                                e_tiles[p][:, half * 512:(half + 1) * 512],
                                start=(kt == 0), stop=(kt == KT - 1),
                            )
                    nc.vector.tensor_mul(ot[h][:, qb * 512:(qb + 1) * 512],
                                         psot[:], rcp[:])

                # partial c_proj, one token block behind the attention
                # pipeline so its weights (ot slices) are long since ready
                for qbp in ([qb - 1] if qb > 0 else []) + (
                        [QB - 1] if qb == QB - 1 else []):
                    for t in range(qbp * 4, qbp * 4 + 4):
                        for nb in range(4):
                            psp = p2ps.tile([128, 512], F32,
                                            tag=f"psp{nb % 2}",
                                            bufs=1, name=f"psp{nb % 2}")
                            for h in range(HL):
                                nc.tensor.matmul(
                                    psp[:],
                                    ot[h][:, t * 128:(t + 1) * 128],
                                    wp_sb[h][:, nb * 512:(nb + 1) * 512],
                                    start=(h == 0), stop=(h == HL - 1),
                                    skip_group_check=True,
                                )
                            st = p2.tile([128, 512], F32, tag="stage", bufs=6,
                                         name="stage")
                            nc.scalar.copy(st[:], psp[:])
                            nc.sync.dma_start(
                                out_ap[t * 128:(t + 1) * 128,
                                       nb * 512:(nb + 1) * 512], st[:])

    nc.compile()
    nc.m = get_hw_module(nc.m)
    return nc


def kernel(hidden_states, attention_mask, w_attn, b_attn, w_proj, b_proj):
    global LAST_RESULTS
    hidden_states = np.asarray(hidden_states, dtype=np.float32)
    attention_mask = np.asarray(attention_mask, dtype=np.float32)
    w_attn = np.asarray(w_attn, dtype=np.float32)
    b_attn = np.asarray(b_attn, dtype=np.float32)
    w_proj = np.asarray(w_proj, dtype=np.float32)
    b_proj = np.asarray(b_proj, dtype=np.float32)

    use_mask = bool((attention_mask != 1.0).any())
    key = ("prog", use_mask)
    if key not in _PROG_CACHE:
        _PROG_CACHE[key] = _build_program(use_mask)
    nc = _PROG_CACHE[key]

    in_maps = []
    for c in range(N_CORES):
        b, g = divmod(c, 4)
        xt = np.ascontiguousarray(hidden_states[b].T).astype(NPF16)
        wq = w_attn[:, g * FL:(g + 1) * FL]
        wk = w_attn[:, D + g * FL:D + (g + 1) * FL]
        wv = w_attn[:, 2 * D + g * FL:2 * D + (g + 1) * FL]
        wqk = np.concatenate([wq, wk], axis=1).astype(NPF16)
        wp = np.ascontiguousarray(w_proj[g * FL:(g + 1) * FL, :]).astype(NPF16)
        bq = b_attn[g * FL:(g + 1) * FL]
        bk = b_attn[D + g * FL:D + (g + 1) * FL]
        bqk = np.ascontiguousarray(
            np.concatenate([bq, bk]).reshape(8, 128).T).astype(np.float32)
        kb = np.ascontiguousarray(
            attention_mask[b].reshape(KT, 128).T).astype(np.float32)
        in_maps.append({
            "xt": xt,
            "wqk": wqk,
            "wv": np.ascontiguousarray(wv).astype(NPF16),
            "wp": wp,
            "bqk": bqk,
            "kb": kb,
        })

    if not os.environ.get("KERNEL_ALLOW_TRACE"):
        os.environ["BASS_NEVER_TRACE"] = "1"
    res = bass_utils.run_bass_kernel_spmd(nc, in_maps, list(range(N_CORES)))
    LAST_RESULTS = res

    # host reduce: sum the 4 head-group partials per batch, add biases.
    # V-bias contribution: rows of A sum to 1, so each core's O gains b_v
    # per row; through c_proj that's a constant row b_v @ w_proj_slice.
    out = np.zeros((B, S, D), dtype=np.float32)
    for c in range(N_CORES):
        b, g = divmod(c, 4)
        out[b] += res.results[c]["out"]
    bias_row = b_proj.astype(np.float64).copy()
    for g in range(4):
        bv = b_attn[2 * D + g * FL:2 * D + (g + 1) * FL].astype(np.float64)
        bias_row += bv @ w_proj[g * FL:(g + 1) * FL, :].astype(np.float64)
    out += bias_row.astype(np.float32)[None, None, :]
    return out
